# revision 4
# baseline (speedup 1.0000x reference)
"""Trainium2 Bass kernel for nn_Basic_Operator_59365037965641.

out = w0*(x+y) + w1*x*y + w2*x/(|y|+eps) + w3*y/(|x|+eps)
    + w4*x*sin(y) + w5*y*sin(x),   w = softmax(param,0).sum(1)

Factored: out = x*A(y) + y*B(x),
    A(y) = w0 + w1*y + w2*g(y) + w4*sin(y),   g(t) = 1/(|t|+eps)
    B(x) = w0 + w3*g(x) + w5*sin(x)

Engine split per [128, F] tile (memory roofline ~268us/core):
  DVE : xr/yr = range-wrap into [-pi,pi] (custom ADD_RANGE_WRAP)
        ax/ay = |t|+eps (custom ABS_ADD_SCALE, 2x perf mode)
        gx/gy = reciprocal_approx_fast -> f32r
  ACT : s_x/s_y = Sin -> f32r; evac psum_A/B (+w0 bias) -> f32r; evac psum_out
  PE  : psum_A = w1*y + w2*gy + w4*s_y ; psum_B = w3*gx + w5*s_x ;
        psum_out = P1 + P2          (all fp32r diag matmuls)
  GP  : P1 = x * A_sb ; P2 = y * B_sb  (tensor_tensor mult, f32r out)

Data-parallel across 8 cores on the leading dim of x/y (flattened rows).
"""

import sys

import numpy as np

sys.path.insert(0, "/opt/trn_rl_repo")

from contextlib import ExitStack

import concourse.bass as bass
import concourse.tile as tile
from concourse import bacc, mybir

PI = float(np.pi)
TWO_PI = float(2.0 * np.pi)
EPS = 1e-8

N_CORES = 8
FULL_ROWS = 16384            # 4*4096
COLS = 4096
SHARD_ROWS = FULL_ROWS // N_CORES       # 2048
P = 128
F_TILE = 2048                            # columns per [128, F] tile
ELEMS = SHARD_ROWS * COLS                # 8M per core
N_TILES = ELEMS // (P * F_TILE)          # 32
F_CHUNK = 512                            # matmul moving-dim max (fp32r)
SLAB = 1024                              # psum slab = 2 chunks, one evac inst

f32 = mybir.dt.float32
f32r = mybir.dt.float32r
Alu = mybir.AluOpType
Act = mybir.ActivationFunctionType

_cached = {}


def _register_abs_add_scale():
    import concourse.dve_ops as D
    from concourse.dve_ops import DveOp, Spec
    from concourse.dve_spec import Src0, C0, C1, C2, maxx

    name = "ABS_ADD_SCALE_P"
    if name in D._SUB_OPCODE_FOR_NAME:
        return [o for o in D.OPS if o.name == name][0]
    op = DveOp(
        name,
        Spec(
            body=(maxx(Src0, Src0 * C2) + C0) * C1,
            reference=lambda in0, in1, c0, c1, c2: (
                (np.maximum(in0.astype(np.float32), in0.astype(np.float32) * c2) + c0)
                * c1
            ),
        ),
        subdim=False,
        uops_sha={},
        perf_en={"v3": True, "v4": True},
    )
    D.OPS.append(op)
    D._SUB_OPCODE_FOR_NAME[op.name] = D._CUSTOM_DVE_ROW_BASE + len(D.OPS) - 1
    D.CUSTOM_DVE_SPECS[op.name] = op.spec
    import re

    for ver in ("v3", "v4"):
        try:
            op.compile(ver)
        except ValueError as e:
            m = re.search(rf"{ver}: ([0-9a-f]+)", str(e))
            op.uops_sha[ver] = m.group(1)
    op.compile("v3")
    return op


def build_bass(w0):
    """Build the Bass program. Only w0 is baked into instructions (ACT evac
    bias); the other weights arrive via the diags input tensor."""
    op_abs = _register_abs_add_scale()
    from concourse.dve_ops import RECIPROCAL_APPROX_FAST, RECIP_APPROX_FAST_CONSTS

    rc = RECIP_APPROX_FAST_CONSTS

    nc = bacc.Bacc("TRN2", target_bir_lowering=False, debug=False)

    x_d = nc.dram_tensor("x", [SHARD_ROWS, COLS], f32, kind="ExternalInput")
    y_d = nc.dram_tensor("y", [SHARD_ROWS, COLS], f32, kind="ExternalInput")
    # 6 stacked [128,128] diagonal matrices: w1, w2, w4, w3, w5, 1.0
    dg_d = nc.dram_tensor("diags", [P, 6 * P], f32, kind="ExternalInput")
    out_d = nc.dram_tensor("out", [SHARD_ROWS, COLS], f32, kind="ExternalOutput")

    xv = x_d.ap().rearrange("(n p) c -> n p c", p=P)   # [8, 128, 4096]
    yv = y_d.ap().rearrange("(n p) c -> n p c", p=P)
    ov = out_d.ap().rearrange("(n p) c -> n p c", p=P)
    row_tiles = xv.shape[0]                 # 16
    col_tiles = COLS // F_TILE              # 2

    with tile.TileContext(nc) as tc, ExitStack() as ctx:
        const_pool = ctx.enter_context(tc.tile_pool(name="const", bufs=1))
        io_pool = ctx.enter_context(tc.tile_pool(name="io", bufs=2))
        wr_pool = ctx.enter_context(tc.tile_pool(name="wr", bufs=1))
        aa_pool = ctx.enter_context(tc.tile_pool(name="aa", bufs=1))
        mid_pool = ctx.enter_context(tc.tile_pool(name="mid", bufs=2))
        ab_pool = ctx.enter_context(tc.tile_pool(name="ab", bufs=2))
        out_pool = ctx.enter_context(tc.tile_pool(name="outp", bufs=2))
        ps_pool = ctx.enter_context(tc.tile_pool(name="ps", bufs=4, space="PSUM"))

        diags = const_pool.tile([P, 6 * P], f32r)
        nc.sync.dma_start(diags[:], dg_d.ap().bitcast(f32r))
        d_w1 = diags[:, 0 * P : 1 * P]
        d_w2 = diags[:, 1 * P : 2 * P]
        d_w4 = diags[:, 2 * P : 3 * P]
        d_w3 = diags[:, 3 * P : 4 * P]
        d_w5 = diags[:, 4 * P : 5 * P]
        d_1 = diags[:, 5 * P : 6 * P]

        n_slabs = F_TILE // SLAB   # 2
        for r in range(row_tiles):
            for cidx in range(col_tiles):
                csl = slice(cidx * F_TILE, (cidx + 1) * F_TILE)
                x_t = io_pool.tile([P, F_TILE], f32r, tag="x")
                nc.sync.dma_start(x_t[:], xv[r][:, csl].bitcast(f32r))
                y_t = io_pool.tile([P, F_TILE], f32r, tag="y")
                nc.sync.dma_start(y_t[:], yv[r][:, csl].bitcast(f32r))
                x_f = x_t[:].bitcast(f32)
                y_f = y_t[:].bitcast(f32)

                # --- DVE preps ---
                xr = wr_pool.tile([P, F_TILE], f32, tag="xr")
                nc.vector.add_range_wrap(xr[:], x_f, 0.0, PI, TWO_PI)
                yr = wr_pool.tile([P, F_TILE], f32, tag="yr")
                nc.vector.add_range_wrap(yr[:], y_f, 0.0, PI, TWO_PI)
                ax = aa_pool.tile([P, F_TILE], f32, tag="aa")
                nc.vector._custom_dve(op_abs, out=ax[:], in0=x_f, s0=EPS, s1=1.0, imm2=-1.0)
                ay = aa_pool.tile([P, F_TILE], f32, tag="aa")
                nc.vector._custom_dve(op_abs, out=ay[:], in0=y_f, s0=EPS, s1=1.0, imm2=-1.0)
                gx = mid_pool.tile([P, F_TILE], f32r, tag="gx")
                nc.vector._custom_dve(
                    RECIPROCAL_APPROX_FAST, out=gx[:], in0=ax[:],
                    s0=rc["s0"], s1=rc["s1"], imm2=rc["imm2"],
                )
                gy = mid_pool.tile([P, F_TILE], f32r, tag="gy")
                nc.vector._custom_dve(
                    RECIPROCAL_APPROX_FAST, out=gy[:], in0=ay[:],
                    s0=rc["s0"], s1=rc["s1"], imm2=rc["imm2"],
                )

                # --- ACT sins ---
                s_x = mid_pool.tile([P, F_TILE], f32r, tag="sx")
                nc.scalar.activation(s_x[:], xr[:], Act.Sin)
                s_y = mid_pool.tile([P, F_TILE], f32r, tag="sy")
                nc.scalar.activation(s_y[:], yr[:], Act.Sin)

                # --- PE sums + ACT evacs ---
                A_sb = ab_pool.tile([P, F_TILE], f32r, tag="A")
                B_sb = ab_pool.tile([P, F_TILE], f32r, tag="B")
                for s in range(n_slabs):
                    ssl = slice(s * SLAB, (s + 1) * SLAB)
                    psA = ps_pool.tile([P, SLAB], f32, tag="ps")
                    for c in range(SLAB // F_CHUNK):
                        cs = slice(s * SLAB + c * F_CHUNK, s * SLAB + (c + 1) * F_CHUNK)
                        pcs = slice(c * F_CHUNK, (c + 1) * F_CHUNK)
                        nc.tensor.matmul(psA[:, pcs], d_w1, y_t[:, cs], start=True, stop=False)
                        nc.tensor.matmul(psA[:, pcs], d_w2, gy[:, cs], start=False, stop=False)
                        nc.tensor.matmul(psA[:, pcs], d_w4, s_y[:, cs], start=False, stop=True)
                    nc.scalar.activation(A_sb[:, ssl], psA[:], Act.Copy, bias=w0, scale=1.0)

                    psB = ps_pool.tile([P, SLAB], f32, tag="ps")
                    for c in range(SLAB // F_CHUNK):
                        cs = slice(s * SLAB + c * F_CHUNK, s * SLAB + (c + 1) * F_CHUNK)
                        pcs = slice(c * F_CHUNK, (c + 1) * F_CHUNK)
                        nc.tensor.matmul(psB[:, pcs], d_w3, gx[:, cs], start=True, stop=False)
                        nc.tensor.matmul(psB[:, pcs], d_w5, s_x[:, cs], start=False, stop=True)
                    nc.scalar.activation(B_sb[:, ssl], psB[:], Act.Copy, bias=w0, scale=1.0)

                # --- GP products ---
                p1 = mid_pool.tile([P, F_TILE], f32r, tag="p1")
                nc.gpsimd.tensor_tensor(p1[:], x_f, A_sb[:].bitcast(f32), Alu.mult)
                p2 = mid_pool.tile([P, F_TILE], f32r, tag="p2")
                nc.gpsimd.tensor_tensor(p2[:], y_f, B_sb[:].bitcast(f32), Alu.mult)

                # --- PE final sum + ACT evac ---
                o_t = out_pool.tile([P, F_TILE], f32, tag="o")
                for s in range(n_slabs):
                    ssl = slice(s * SLAB, (s + 1) * SLAB)
                    psO = ps_pool.tile([P, SLAB], f32, tag="ps")
                    for c in range(SLAB // F_CHUNK):
                        cs = slice(s * SLAB + c * F_CHUNK, s * SLAB + (c + 1) * F_CHUNK)
                        pcs = slice(c * F_CHUNK, (c + 1) * F_CHUNK)
                        nc.tensor.matmul(psO[:, pcs], d_1, p1[:, cs], start=True, stop=False)
                        nc.tensor.matmul(psO[:, pcs], d_1, p2[:, cs], start=False, stop=True)
                    nc.scalar.activation(o_t[:, ssl], psO[:], Act.Copy, bias=0.0, scale=1.0)

                nc.sync.dma_start(ov[r][:, csl], o_t[:])

    nc.finalize()
    return nc


def _get_program(w0):
    key = float(np.float32(w0))
    if key not in _cached:
        _cached[key] = build_bass(key)
    return _cached[key]


def _weights(param):
    param = np.asarray(param, dtype=np.float64)
    m = param.max(axis=0, keepdims=True)
    e = np.exp(param - m)
    soft = e / e.sum(axis=0, keepdims=True)
    return soft.sum(axis=1)  # [6]


def _diags(w):
    eye = np.eye(P, dtype=np.float32)
    order = [w[1], w[2], w[4], w[3], w[5], 1.0]
    return np.concatenate([eye * np.float32(v) for v in order], axis=1).astype(np.float32)


def _run(x, y, param, trace=False):
    from concourse.bass_utils import run_bass_kernel_spmd

    x = np.asarray(x)
    y = np.asarray(y)
    w = _weights(param)
    nc = _get_program(w[0])

    xf = np.ascontiguousarray(x.reshape(FULL_ROWS, COLS))
    yf = np.ascontiguousarray(y.reshape(FULL_ROWS, COLS))
    dg = _diags(w)

    in_maps = []
    for c in range(N_CORES):
        rows = slice(c * SHARD_ROWS, (c + 1) * SHARD_ROWS)
        in_maps.append({"x": xf[rows], "y": yf[rows], "diags": dg})

    res = run_bass_kernel_spmd(
        nc, in_maps, core_ids=list(range(N_CORES)), trace=trace
    )
    out = np.empty((FULL_ROWS, COLS), dtype=np.float32)
    for c in range(N_CORES):
        out[c * SHARD_ROWS : (c + 1) * SHARD_ROWS] = res.results[c]["out"]
    return out.reshape(x.shape), res


def kernel(x, y, param):
    out, _ = _run(x, y, param, trace=False)
    return out


def kernel_traced(x, y, param):
    """Run with NTFF tracing; returns exec_time_ns (or None)."""
    out, res = _run(x, y, param, trace=True)
    return res.exec_time_ns
